# revision 5
# baseline (speedup 1.0000x reference)
"""ECG spiking encoder (conv-tokenizer + 2x {linear, parametric-LIF} + time-mean)
as a Bass kernel on 8 TRN2 NeuronCores, pure data parallel over batch.

Math (per core, batch shard of 64):
  patches   = im2col(x)                      # stride==kernel -> pure relayout
  h1        = patches @ Wc.T + bc            # conv fused with fc1 (host weight fold)
  u1        = sig1*h1 + sig1*bc              # folded into GEMM weights + epilogue bias
  LIF1      : v <- v + (h1 - v)*sig1 ; s = H(v-1) ; v <- v - s
  h2/u2     = fc2(s1) ...
  LIF2      ; out = mean_t(s2)

Device mapping:
  GEMM1: bf16 hi/lo 3-pass (exact to ~2^-16) over k=640 (5 chunks of 128, zero pad)
  GEMM2: float32r
  LIF   : one fused custom DVE op per step, both layers merged in one [128,128] tile
          state v'_t = (v'_{t-1} - (v'_{t-1} > 1)) * a + u_t   (v'-form, u pre-scaled)
  spikes: bulk  s = (v' > 1)  extraction; mean via tensor_reduce over t.
"""
import numpy as np
import ml_dtypes
from contextlib import ExitStack

import concourse.bass as bass
import concourse.tile as tile
from concourse import bacc, mybir
from concourse.bass_utils import run_bass_kernel_spmd

F32 = mybir.dt.float32
F32R = mybir.dt.float32r
BF16 = mybir.dt.bfloat16

# ---- problem constants (hardcoded per contract) ----
B, C, L = 512, 12, 5000
E, H1, H2, P = 128, 128, 128, 50
T = 100
STRIDE = 50
V_TH = 1.0
NCORES = 8
BS = B // NCORES          # 64 batch per core
K = C * P                 # 600 contraction
KPAD = 640                # 5 chunks of 128
NCH = KPAD // 128         # 5
NT = 13                   # row tiles: 12x512 + 1x256
ROWS = T * BS             # 6400
LAG = 8                   # layer-2 lags layer-1 by 8 steps (one block)
NBLK = T // 8             # 12.5 -> handled as 13 blocks (last half)
MSTEPS = T + LAG          # 108 merged scan steps


def _register_lif_op():
    """Fused LIF step as a custom DVE op, via the documented extension point
    (concourse dve_ops registry). Idempotent across calls."""
    import concourse.dve_ops as dom
    from concourse.dve_spec import Spec, Src0, Src1, C0, C1, lower, _has_src1
    from concourse.dve_uop import DveOpSpec

    name = "LIF_EMA_RESET_STEP"
    for op in dom.OPS:
        if op.name == name:
            return op

    body = (Src0 - (Src0 > C1)) * C0 + Src1

    def ref(in0, in1, s0, s1, imm2):
        return (((in0 - (in0 > s1)) * s0) + in1).astype(np.float32)

    spec = Spec(body=body, reference=ref)
    row = dom._CUSTOM_DVE_ROW_BASE + len(dom.OPS)
    assert row < 0x20
    shas = {}
    for ver in ("v3", "v4"):
        uops = lower(spec, ver=ver)
        shas[ver] = DveOpSpec(name=name, opcode=row, uops=uops,
                              rd1_en=_has_src1(spec)).sha(ver)
    op = dom.DveOp(name, spec, subdim=False, uops_sha=shas)
    dom.OPS.append(op)
    dom._SUB_OPCODE_FOR_NAME[name] = row
    dom.CUSTOM_DVE_SPECS[name] = spec
    return op


def _build_program(a1: float, a2: float):
    lif_op = _register_lif_op()
    nc = bacc.Bacc("TRN2", target_bir_lowering=False, debug=False,
                   num_devices=NCORES)

    xhl_d = nc.dram_tensor("xhl", [2 * NCH, 128, ROWS], BF16, kind="ExternalInput").ap()
    w1_d = nc.dram_tensor("w1", [2 * NCH, 128, H1], BF16, kind="ExternalInput").ap()
    b1_d = nc.dram_tensor("b1", [128, 1], F32, kind="ExternalInput").ap()
    w2_d = nc.dram_tensor("w2", [128, H2], F32R, kind="ExternalInput").ap()
    b2_d = nc.dram_tensor("b2", [128, 1], F32, kind="ExternalInput").ap()
    out_d = nc.dram_tensor("out", [128, BS], F32, kind="ExternalOutput").ap()

    # GEMM1 3-pass MM schedule: hi*Wh (5), lo*Wh (5), hi*Wl (5)
    W_IDX = [0, 1, 2, 3, 4] + [0, 1, 2, 3, 4] + [5, 6, 7, 8, 9]
    X_IDX = [0, 1, 2, 3, 4] + [5, 6, 7, 8, 9] + [0, 1, 2, 3, 4]

    GROUPS = [(0, 2048), (2048, 2048), (4096, 2048), (6144, 256)]  # col ranges

    with tile.TileContext(nc) as tc, ExitStack() as ctx:
        wpool = ctx.enter_context(tc.tile_pool(name="wpool", bufs=1))
        xpool = ctx.enter_context(tc.tile_pool(name="xpool", bufs=2))
        upool = ctx.enter_context(tc.tile_pool(name="upool", bufs=4))
        spool = ctx.enter_context(tc.tile_pool(name="spool", bufs=3))
        vpool = ctx.enter_context(tc.tile_pool(name="vpool", bufs=1))
        ps1pool = ctx.enter_context(tc.tile_pool(name="ps1", bufs=4, space="PSUM"))
        ps2pool = ctx.enter_context(tc.tile_pool(name="ps2", bufs=2, space="PSUM"))
        mpool = ctx.enter_context(tc.tile_pool(name="mpool", bufs=1))

        # weights
        wt = wpool.tile([128, 10 * H1], BF16)
        for i in range(10):
            nc.sync.dma_start(wt[:, bass.ts(i, H1)], w1_d[i])
        w2t = wpool.tile([128, H2], F32R)
        nc.sync.dma_start(w2t[:], w2_d[:])
        b1t = wpool.tile([128, 1], F32)
        nc.sync.dma_start(b1t[:], b1_d[:])
        b2t = wpool.tile([128, 1], F32)
        nc.sync.dma_start(b2t[:], b2_d[:])

        # big state buffer: merged v' trajectory [128, MSTEPS*128]
        vball = vpool.tile([128, MSTEPS * 128], F32)
        zinit = wpool.tile([128, 128], F32)
        nc.vector.memset(zinit[:], 0.0)

        # u blocks: 14 logical blocks (m = 8k..8k+7), rolling pool
        ublks = [None] * 14

        def ublk_for(k):
            if ublks[k] is None:
                t_ = upool.tile([128, 8 * 128], F32, tag="ublk", name=f"ublk{k}")
                ublks[k] = t_
            return ublks[k]

        s1blks = [None] * NT

        m_done = 0  # merged scan steps emitted

        merged = float(a1) == float(a2)

        def emit_scan_through(m_end):
            nonlocal m_done
            while m_done < m_end:
                m = m_done
                ub = ublks[m // 8]
                src = zinit[:] if m == 0 else vball[:, bass.ts(m - 1, 128)]
                if merged:
                    nc.vector._custom_dve(
                        lif_op, out=vball[:, bass.ts(m, 128)], in0=src,
                        in1=ub[:, bass.ts(m % 8, 128)], s0=a1, s1=V_TH)
                else:
                    nc.vector._custom_dve(
                        lif_op, out=vball[:, m * 128:m * 128 + 64],
                        in0=src[:, 0:64] if m == 0 else vball[:, (m - 1) * 128:(m - 1) * 128 + 64],
                        in1=ub[:, (m % 8) * 128:(m % 8) * 128 + 64], s0=a1, s1=V_TH)
                    nc.vector._custom_dve(
                        lif_op, out=vball[:, m * 128 + 64:m * 128 + 128],
                        in0=src[:, 64:128] if m == 0 else vball[:, (m - 1) * 128 + 64:m * 128],
                        in1=ub[:, (m % 8) * 128 + 64:(m % 8) * 128 + 128], s0=a2, s1=V_TH)
                m_done += 1

        xg = None
        gi = -1

        for j in range(NT):  # row tiles (8 t-steps each; last tile = 4)
            col0 = j * 512
            ncols = 512 if j < 12 else 256
            nsteps = ncols // 64

            # DMA the x column-group when entering a new group
            g = min(col0 // 2048, 3)
            if g != gi:
                gi = g
                gc0, gcn = GROUPS[g]
                xg = xpool.tile([128, 10 * 2048], BF16, tag="xg")
                for i in range(10):
                    nc.sync.dma_start(xg[:, i * 2048:i * 2048 + gcn],
                                      xhl_d[i, :, gc0:gc0 + gcn])
            goff = col0 - GROUPS[g][0]

            # GEMM1: 15 bf16 matmuls accumulating in one PSUM bank
            ps = ps1pool.tile([128, ncols], F32, tag="ps1t")
            for i in range(15):
                nc.tensor.matmul(
                    ps[:], wt[:, bass.ts(W_IDX[i], H1)],
                    xg[:, X_IDX[i] * 2048 + goff: X_IDX[i] * 2048 + goff + ncols],
                    start=(i == 0), stop=(i == 14))

            # epilogue 1: u1 <- ps + b1 into L1 halves of u block j
            ub = ublk_for(j)
            if j == 0:
                nc.vector.memset(ub[:], 0.0)  # L2 halves of block 0 = 0
            nc.scalar.activation(
                ub[:].rearrange("p (s c) -> p s c", c=128)[:, :nsteps, 0:64],
                ps[:].rearrange("p (s c) -> p s c", c=64),
                mybir.ActivationFunctionType.Identity, bias=b1t[:, 0:1])
            if j == 12:
                # L1 halves of merged steps 100..107 get u=0
                ub13 = ublk_for(13)
                nc.vector.memset(ub13[:], 0.0)
                nc.vector.memset(
                    ub[:].rearrange("p (s c) -> p s c", c=128)[:, nsteps:, 0:64], 0.0)

            # scan merged steps for block j (needs u-block j complete: L1 from
            # epi1(j) above, L2 from epi2(j-1) emitted last iteration)
            emit_scan_through(min(8 * (j + 1), MSTEPS))

            # s1 extraction for block j (v' of merged steps 8j..8j+8)
            sb = spool.tile([128, 512], F32R, tag="s1b", name=f"s1b{j}")
            s1blks[j] = sb
            nc.vector.tensor_scalar(
                sb[:].rearrange("p (s c) -> p s c", c=64)[:, :nsteps],
                vball[:].rearrange("p (m c) -> p m c", c=128)[:, 8 * j:8 * j + nsteps, 0:64],
                V_TH, None, mybir.AluOpType.is_gt, mybir.AluOpType.bypass)

            # GEMM2 on block j spikes -> u2 for L2 steps 8j..; lands in u block j+1
            ps2 = ps2pool.tile([128, ncols], F32, tag="ps2t")
            nc.tensor.matmul(ps2[:], w2t[:], sb[:, :ncols], start=True, stop=True)
            ub_next = ublk_for(j + 1)
            nc.scalar.activation(
                ub_next[:].rearrange("p (s c) -> p s c", c=128)[:, :nsteps, 64:128],
                ps2[:].rearrange("p (s c) -> p s c", c=64),
                mybir.ActivationFunctionType.Identity, bias=b2t[:, 0:1])
            if j == 12 and nsteps < 8:
                nc.vector.memset(
                    ub_next[:].rearrange("p (s c) -> p s c", c=128)[:, nsteps:, 64:128], 0.0)

        # flush remaining merged steps (L2 tail)
        emit_scan_through(MSTEPS)

        # layer-2 spikes + mean over t: v'2 for L2 t=0..99 lives at merged
        # steps m=8..107, columns 64:128 of each block
        v2view = vball[:].rearrange("p (m c) -> p m c", c=128)[:, LAG:LAG + T, 64:128]
        nc.vector.tensor_scalar(v2view, v2view, V_TH, None,
                                mybir.AluOpType.is_gt, mybir.AluOpType.bypass)
        acc = mpool.tile([128, BS], F32)
        v2bt = vball[:].rearrange("p (m c) -> p c m", c=128)[:, 64:128, LAG:LAG + T]
        nc.vector.tensor_reduce(acc[:], v2bt, mybir.AxisListType.X,
                                mybir.AluOpType.add)
        nc.vector.tensor_scalar(acc[:], acc[:], float(np.float32(1.0 / T)), None,
                                mybir.AluOpType.mult, mybir.AluOpType.bypass)
        nc.sync.dma_start(out_d[:], acc[:])

    nc.compile()
    return nc


_PROG_CACHE = {}


def _get_program(a1, a2):
    key = (round(float(a1), 10), round(float(a2), 10))
    if key not in _PROG_CACHE:
        _PROG_CACHE[key] = _build_program(float(a1), float(a2))
    return _PROG_CACHE[key]


def kernel(x, conv_w, conv_b, fc1_w, fc1_b, fc2_w, fc2_b, w1, w2):
    x = np.asarray(x, np.float32)
    conv_w = np.asarray(conv_w, np.float32)
    conv_b = np.asarray(conv_b, np.float32)
    fc1_w = np.asarray(fc1_w, np.float32)
    fc1_b = np.asarray(fc1_b, np.float32)
    fc2_w = np.asarray(fc2_w, np.float32)
    fc2_b = np.asarray(fc2_b, np.float32)

    sig1 = 1.0 / (1.0 + np.exp(-np.float64(w1)))
    sig2 = 1.0 / (1.0 + np.exp(-np.float64(w2)))
    a1 = np.float32(1.0 - sig1)
    a2 = np.float32(1.0 - sig2)
    sig1 = np.float32(sig1)
    sig2 = np.float32(sig2)

    # ---- weight folding (host, fp64 for exactness headroom) ----
    # u1 = sig1*(fc1_w @ (conv_w.x + conv_b) + fc1_b)
    Wc = sig1.astype(np.float64) * (fc1_w.astype(np.float64) @ conv_w.reshape(E, K).astype(np.float64))
    bc = sig1.astype(np.float64) * (fc1_w.astype(np.float64) @ conv_b.astype(np.float64) + fc1_b.astype(np.float64))
    Wc = Wc.astype(np.float32)                      # [H1, K]
    bc = bc.astype(np.float32)                      # [H1]
    Wcp = np.zeros((H1, KPAD), np.float32)
    Wcp[:, :K] = Wc
    # lhsT chunks [k, H1], bf16 hi/lo
    WcT = Wcp.T.copy()                              # [KPAD, H1]
    Wh = WcT.astype(ml_dtypes.bfloat16)
    Wl = (WcT - Wh.astype(np.float32)).astype(ml_dtypes.bfloat16)
    w1_arr = np.concatenate([
        Wh.reshape(NCH, 128, H1), Wl.reshape(NCH, 128, H1)], axis=0)  # [10,128,H1]

    W2T = (sig2.astype(np.float64) * fc2_w.astype(np.float64)).T.astype(np.float32).copy()  # [H1, H2] lhsT
    b1_arr = bc.reshape(128, 1)
    b2_arr = (sig2 * fc2_b).astype(np.float32).reshape(128, 1)

    # ---- im2col + shard (pure relayout; stride == kernel width) ----
    # x [B, C, L] -> per-core [64, C, T, P] -> (c, p, t, b) -> [K, T*BS]
    in_maps = []
    for ci in range(NCORES):
        xs = x[ci * BS:(ci + 1) * BS].reshape(BS, C, T, P)
        xT = np.ascontiguousarray(xs.transpose(1, 3, 2, 0)).reshape(K, ROWS)
        xTp = np.zeros((KPAD, ROWS), np.float32)
        xTp[:K] = xT
        xh = xTp.astype(ml_dtypes.bfloat16)
        xl = (xTp - xh.astype(np.float32)).astype(ml_dtypes.bfloat16)
        xhl = np.concatenate([xh.reshape(NCH, 128, ROWS),
                              xl.reshape(NCH, 128, ROWS)], axis=0)
        in_maps.append({
            "xhl": xhl, "w1": w1_arr, "b1": b1_arr,
            "w2": W2T, "b2": b2_arr,
        })

    nc = _get_program(a1, a2)
    res = run_bass_kernel_spmd(nc, in_maps, list(range(NCORES)))

    out = np.empty((B, H2), np.float32)
    for ci in range(NCORES):
        out[ci * BS:(ci + 1) * BS] = res.results[ci]["out"].T
    return out


# revision 6
# speedup vs baseline: 1.0541x; 1.0541x over previous
"""ECG spiking encoder (conv-tokenizer + 2x {linear, parametric-LIF} + time-mean)
as a Bass kernel on 8 TRN2 NeuronCores, pure data parallel over batch.

Math (per core, batch shard of 64):
  patches   = im2col(x)                      # stride==kernel -> pure relayout
  h1        = patches @ Wc.T + bc            # conv fused with fc1 (host weight fold)
  u1        = sig1*h1 + sig1*bc              # folded into GEMM weights + epilogue bias
  LIF1      : v <- v + (h1 - v)*sig1 ; s = H(v-1) ; v <- v - s
  h2/u2     = fc2(s1) ...
  LIF2      ; out = mean_t(s2)

Device mapping:
  GEMM1: bf16 hi/lo 3-pass (exact to ~2^-16) over k=640 (5 chunks of 128, zero pad)
  GEMM2: float32r
  LIF   : one fused custom DVE op per step, both layers merged in one [128,128] tile
          state v'_t = (v'_{t-1} - (v'_{t-1} > 1)) * a + u_t   (v'-form, u pre-scaled)
  spikes: bulk  s = (v' > 1)  extraction; mean via tensor_reduce over t.
"""
import numpy as np
import ml_dtypes
from contextlib import ExitStack

import concourse.bass as bass
import concourse.tile as tile
from concourse import bacc, mybir
from concourse.bass_utils import run_bass_kernel_spmd

F32 = mybir.dt.float32
F32R = mybir.dt.float32r
BF16 = mybir.dt.bfloat16

# ---- problem constants (hardcoded per contract) ----
B, C, L = 512, 12, 5000
E, H1, H2, P = 128, 128, 128, 50
T = 100
STRIDE = 50
V_TH = 1.0
NCORES = 8
BS = B // NCORES          # 64 batch per core
K = C * P                 # 600 contraction
KPAD = 640                # 5 chunks of 128
NCH = KPAD // 128         # 5
NT = 13                   # row tiles: 12x512 + 1x256
ROWS = T * BS             # 6400
LAG = 16                  # layer-2 lags layer-1 by 16 steps (two blocks)
NBLK = T // 8             # 12.5 -> handled as 13 blocks (last half)
MSTEPS = T + LAG          # 108 merged scan steps


def _register_lif_op():
    """Fused LIF step as a custom DVE op, via the documented extension point
    (concourse dve_ops registry). Idempotent across calls."""
    import concourse.dve_ops as dom
    from concourse.dve_spec import Spec, Src0, Src1, C0, C1, lower, _has_src1
    from concourse.dve_uop import DveOpSpec

    name = "LIF_EMA_RESET_STEP"
    for op in dom.OPS:
        if op.name == name:
            return op

    body = (Src0 - (Src0 > C1)) * C0 + Src1

    def ref(in0, in1, s0, s1, imm2):
        return (((in0 - (in0 > s1)) * s0) + in1).astype(np.float32)

    spec = Spec(body=body, reference=ref)
    row = dom._CUSTOM_DVE_ROW_BASE + len(dom.OPS)
    assert row < 0x20
    shas = {}
    for ver in ("v3", "v4"):
        uops = lower(spec, ver=ver)
        shas[ver] = DveOpSpec(name=name, opcode=row, uops=uops,
                              rd1_en=_has_src1(spec)).sha(ver)
    op = dom.DveOp(name, spec, subdim=False, uops_sha=shas)
    dom.OPS.append(op)
    dom._SUB_OPCODE_FOR_NAME[name] = row
    dom.CUSTOM_DVE_SPECS[name] = spec
    return op


def _build_program(a1: float, a2: float):
    lif_op = _register_lif_op()
    nc = bacc.Bacc("TRN2", target_bir_lowering=False, debug=False,
                   num_devices=NCORES)

    xhl_d = nc.dram_tensor("xhl", [2 * NCH, 128, ROWS], BF16, kind="ExternalInput").ap()
    w1_d = nc.dram_tensor("w1", [2 * NCH, 128, H1], BF16, kind="ExternalInput").ap()
    b1_d = nc.dram_tensor("b1", [128, 1], F32, kind="ExternalInput").ap()
    w2_d = nc.dram_tensor("w2", [128, H2], F32R, kind="ExternalInput").ap()
    b2_d = nc.dram_tensor("b2", [128, 1], F32, kind="ExternalInput").ap()
    out_d = nc.dram_tensor("out", [128, BS], F32, kind="ExternalOutput").ap()

    # GEMM1 3-pass MM schedule: hi*Wh (5), lo*Wh (5), hi*Wl (5)
    W_IDX = [0, 1, 2, 3, 4] + [0, 1, 2, 3, 4] + [5, 6, 7, 8, 9]
    X_IDX = [0, 1, 2, 3, 4] + [5, 6, 7, 8, 9] + [0, 1, 2, 3, 4]

    GROUPS = [(0, 2048), (2048, 2048), (4096, 2048), (6144, 256)]  # col ranges

    with tile.TileContext(nc) as tc, ExitStack() as ctx:
        wpool = ctx.enter_context(tc.tile_pool(name="wpool", bufs=1))
        xpool = ctx.enter_context(tc.tile_pool(name="xpool", bufs=2))
        upool = ctx.enter_context(tc.tile_pool(name="upool", bufs=6))
        spool = ctx.enter_context(tc.tile_pool(name="spool", bufs=3))
        vpool = ctx.enter_context(tc.tile_pool(name="vpool", bufs=1))
        ps1pool = ctx.enter_context(tc.tile_pool(name="ps1", bufs=4, space="PSUM"))
        ps2pool = ctx.enter_context(tc.tile_pool(name="ps2", bufs=2, space="PSUM"))
        mpool = ctx.enter_context(tc.tile_pool(name="mpool", bufs=1))

        # weights
        wt = wpool.tile([128, 10 * H1], BF16)
        for i in range(10):
            nc.sync.dma_start(wt[:, bass.ts(i, H1)], w1_d[i])
        w2t = wpool.tile([128, H2], F32R)
        nc.sync.dma_start(w2t[:], w2_d[:])
        b1t = wpool.tile([128, 1], F32)
        nc.sync.dma_start(b1t[:], b1_d[:])
        b2t = wpool.tile([128, 1], F32)
        nc.sync.dma_start(b2t[:], b2_d[:])

        # big state buffer: merged v' trajectory [128, MSTEPS*128]
        vball = vpool.tile([128, MSTEPS * 128], F32)
        zinit = wpool.tile([128, 128], F32)
        nc.vector.memset(zinit[:], 0.0)

        # u blocks: 14 logical blocks (m = 8k..8k+7), rolling pool
        ublks = [None] * 15

        def ublk_for(k):
            if ublks[k] is None:
                t_ = upool.tile([128, 8 * 128], F32, tag="ublk", name=f"ublk{k}")
                ublks[k] = t_
            return ublks[k]

        s1blks = [None] * NT

        m_done = 0  # merged scan steps emitted

        merged = float(a1) == float(a2)

        def emit_scan_through(m_end):
            nonlocal m_done
            while m_done < m_end:
                m = m_done
                ub = ublks[m // 8]
                src = zinit[:] if m == 0 else vball[:, bass.ts(m - 1, 128)]
                if merged:
                    nc.vector._custom_dve(
                        lif_op, out=vball[:, bass.ts(m, 128)], in0=src,
                        in1=ub[:, bass.ts(m % 8, 128)], s0=a1, s1=V_TH)
                else:
                    nc.vector._custom_dve(
                        lif_op, out=vball[:, m * 128:m * 128 + 64],
                        in0=src[:, 0:64] if m == 0 else vball[:, (m - 1) * 128:(m - 1) * 128 + 64],
                        in1=ub[:, (m % 8) * 128:(m % 8) * 128 + 64], s0=a1, s1=V_TH)
                    nc.vector._custom_dve(
                        lif_op, out=vball[:, m * 128 + 64:m * 128 + 128],
                        in0=src[:, 64:128] if m == 0 else vball[:, (m - 1) * 128 + 64:m * 128],
                        in1=ub[:, (m % 8) * 128 + 64:(m % 8) * 128 + 128], s0=a2, s1=V_TH)
                m_done += 1

        xg = None
        gi = -1

        for j in range(NT):  # row tiles (8 t-steps each; last tile = 4)
            col0 = j * 512
            ncols = 512 if j < 12 else 256
            nsteps = ncols // 64

            # DMA the x column-group when entering a new group
            g = min(col0 // 2048, 3)
            if g != gi:
                gi = g
                gc0, gcn = GROUPS[g]
                xg = xpool.tile([128, 10 * 2048], BF16, tag="xg")
                for i in range(10):
                    nc.sync.dma_start(xg[:, i * 2048:i * 2048 + gcn],
                                      xhl_d[i, :, gc0:gc0 + gcn])
            goff = col0 - GROUPS[g][0]

            # GEMM1: 15 bf16 matmuls accumulating in one PSUM bank
            ps = ps1pool.tile([128, ncols], F32, tag="ps1t")
            for i in range(15):
                nc.tensor.matmul(
                    ps[:], wt[:, bass.ts(W_IDX[i], H1)],
                    xg[:, X_IDX[i] * 2048 + goff: X_IDX[i] * 2048 + goff + ncols],
                    start=(i == 0), stop=(i == 14))

            # epilogue 1: u1 <- ps + b1 into L1 halves of u block j
            ub = ublk_for(j)
            if j <= 1:
                nc.vector.memset(ub[:], 0.0)  # L2 halves of blocks 0,1 = 0
            nc.scalar.activation(
                ub[:].rearrange("p (s c) -> p s c", c=128)[:, :nsteps, 0:64],
                ps[:].rearrange("p (s c) -> p s c", c=64),
                mybir.ActivationFunctionType.Identity, bias=b1t[:, 0:1])
            if j == 12:
                # L1 halves of merged steps 100..103 get u=0
                nc.vector.memset(
                    ub[:].rearrange("p (s c) -> p s c", c=128)[:, nsteps:, 0:64], 0.0)

            # scan merged steps for block j (needs u-block j complete: L1 from
            # epi1(j) above, L2 from epi2(j-1) emitted last iteration)
            emit_scan_through(min(8 * (j + 1), MSTEPS))

            # s1 extraction for block j (v' of merged steps 8j..8j+8)
            sb = spool.tile([128, 512], F32R, tag="s1b", name=f"s1b{j}")
            s1blks[j] = sb
            nc.vector.tensor_scalar(
                sb[:].rearrange("p (s c) -> p s c", c=64)[:, :nsteps],
                vball[:].rearrange("p (m c) -> p m c", c=128)[:, 8 * j:8 * j + nsteps, 0:64],
                V_TH, None, mybir.AluOpType.is_gt, mybir.AluOpType.bypass)

            # GEMM2 on block j spikes -> u2 for L2 steps 8j..; lands in u block j+1
            ps2 = ps2pool.tile([128, ncols], F32, tag="ps2t")
            nc.tensor.matmul(ps2[:], w2t[:], sb[:, :ncols], start=True, stop=True)
            ub_next = ublk_for(j + 2)
            if j + 2 >= 13:
                # L1 halves of tail blocks (merged steps >= 104) get u=0
                nc.vector.memset(
                    ub_next[:].rearrange("p (s c) -> p s c", c=128)[:, :, 0:64], 0.0)
            nc.scalar.activation(
                ub_next[:].rearrange("p (s c) -> p s c", c=128)[:, :nsteps, 64:128],
                ps2[:].rearrange("p (s c) -> p s c", c=64),
                mybir.ActivationFunctionType.Identity, bias=b2t[:, 0:1])
            if j == 12 and nsteps < 8:
                nc.vector.memset(
                    ub_next[:].rearrange("p (s c) -> p s c", c=128)[:, nsteps:, 64:128], 0.0)

        # flush remaining merged steps (L2 tail)
        emit_scan_through(MSTEPS)

        # layer-2 spikes + mean over t: v'2 for L2 t=0..99 lives at merged
        # steps m=8..107, columns 64:128 of each block
        v2view = vball[:].rearrange("p (m c) -> p m c", c=128)[:, LAG:LAG + T, 64:128]
        nc.vector.tensor_scalar(v2view, v2view, V_TH, None,
                                mybir.AluOpType.is_gt, mybir.AluOpType.bypass)
        acc = mpool.tile([128, BS], F32)
        v2bt = vball[:].rearrange("p (m c) -> p c m", c=128)[:, 64:128, LAG:LAG + T]
        nc.vector.tensor_reduce(acc[:], v2bt, mybir.AxisListType.X,
                                mybir.AluOpType.add)
        nc.vector.tensor_scalar(acc[:], acc[:], float(np.float32(1.0 / T)), None,
                                mybir.AluOpType.mult, mybir.AluOpType.bypass)
        nc.sync.dma_start(out_d[:], acc[:])

    nc.compile()
    return nc


_PROG_CACHE = {}


def _get_program(a1, a2):
    key = (round(float(a1), 10), round(float(a2), 10))
    if key not in _PROG_CACHE:
        _PROG_CACHE[key] = _build_program(float(a1), float(a2))
    return _PROG_CACHE[key]


def kernel(x, conv_w, conv_b, fc1_w, fc1_b, fc2_w, fc2_b, w1, w2):
    x = np.asarray(x, np.float32)
    conv_w = np.asarray(conv_w, np.float32)
    conv_b = np.asarray(conv_b, np.float32)
    fc1_w = np.asarray(fc1_w, np.float32)
    fc1_b = np.asarray(fc1_b, np.float32)
    fc2_w = np.asarray(fc2_w, np.float32)
    fc2_b = np.asarray(fc2_b, np.float32)

    sig1 = 1.0 / (1.0 + np.exp(-np.float64(w1)))
    sig2 = 1.0 / (1.0 + np.exp(-np.float64(w2)))
    a1 = np.float32(1.0 - sig1)
    a2 = np.float32(1.0 - sig2)
    sig1 = np.float32(sig1)
    sig2 = np.float32(sig2)

    # ---- weight folding (host, fp64 for exactness headroom) ----
    # u1 = sig1*(fc1_w @ (conv_w.x + conv_b) + fc1_b)
    Wc = sig1.astype(np.float64) * (fc1_w.astype(np.float64) @ conv_w.reshape(E, K).astype(np.float64))
    bc = sig1.astype(np.float64) * (fc1_w.astype(np.float64) @ conv_b.astype(np.float64) + fc1_b.astype(np.float64))
    Wc = Wc.astype(np.float32)                      # [H1, K]
    bc = bc.astype(np.float32)                      # [H1]
    Wcp = np.zeros((H1, KPAD), np.float32)
    Wcp[:, :K] = Wc
    # lhsT chunks [k, H1], bf16 hi/lo
    WcT = Wcp.T.copy()                              # [KPAD, H1]
    Wh = WcT.astype(ml_dtypes.bfloat16)
    Wl = (WcT - Wh.astype(np.float32)).astype(ml_dtypes.bfloat16)
    w1_arr = np.concatenate([
        Wh.reshape(NCH, 128, H1), Wl.reshape(NCH, 128, H1)], axis=0)  # [10,128,H1]

    W2T = (sig2.astype(np.float64) * fc2_w.astype(np.float64)).T.astype(np.float32).copy()  # [H1, H2] lhsT
    b1_arr = bc.reshape(128, 1)
    b2_arr = (sig2 * fc2_b).astype(np.float32).reshape(128, 1)

    # ---- im2col + shard (pure relayout; stride == kernel width) ----
    # x [B, C, L] -> per-core [64, C, T, P] -> (c, p, t, b) -> [K, T*BS]
    in_maps = []
    for ci in range(NCORES):
        xs = x[ci * BS:(ci + 1) * BS].reshape(BS, C, T, P)
        xT = np.ascontiguousarray(xs.transpose(1, 3, 2, 0)).reshape(K, ROWS)
        xTp = np.zeros((KPAD, ROWS), np.float32)
        xTp[:K] = xT
        xh = xTp.astype(ml_dtypes.bfloat16)
        xl = (xTp - xh.astype(np.float32)).astype(ml_dtypes.bfloat16)
        xhl = np.concatenate([xh.reshape(NCH, 128, ROWS),
                              xl.reshape(NCH, 128, ROWS)], axis=0)
        in_maps.append({
            "xhl": xhl, "w1": w1_arr, "b1": b1_arr,
            "w2": W2T, "b2": b2_arr,
        })

    nc = _get_program(a1, a2)
    res = run_bass_kernel_spmd(nc, in_maps, list(range(NCORES)))

    out = np.empty((B, H2), np.float32)
    for ci in range(NCORES):
        out[ci * BS:(ci + 1) * BS] = res.results[ci]["out"].T
    return out
